# revision 15
# baseline (speedup 1.0000x reference)
"""Per-pixel adaptive 5x5 conv (KPN) for Trainium2, 8-core data parallel.

out[g,h,w] = sum_{i,j} core[g,5i+j,h,w] * frames_pad[g,h+i-2,w+j-2]
with g = flattened (B,N) = 16 image planes; 2 planes per NeuronCore.

Engine split (v8):
  DVE    : the 25 per-tap multiplies per image, ALL in fp16 2x_1P mode,
           plus half of the last image's PSUM casts.
  TensorE: all tap accumulation, via identity-matmul into PSUM (fp32
           accumulate; 4 matmuls of FD=512 per tap = 1 PSUM bank each).
  ScalarE: builds the odd-parity frame copy on-chip (saves 2.1 MB of HBM
           per core), img0's PSUM casts, half the output stores.
  GpSimd : idle (shares SBUF ports with DVE; using it slows DVE).
           5 of 25 tap planes per image ship as raw fp8-e4m3 (halves
           their HBM bytes; those DVE muls run 1x). rel err ~1.2e-2.

The end-to-end time is paced by DVE (25 x 1.22us muls per image) vs the
HBM weight stream through 16 SDMA engines, one of which runs ~17% slow
and gates every completion semaphore; fp8 shipping + the on-chip parity
copy + fp16 outputs cut the stream to ~24 MB/core to rebalance. First
fp8 group streams per-tap (compute starts ~12.5us), last fp16 group
streams per-tap (post-stream tail is one tap), final casts/stores
pipeline with the last taps' matmuls on alternating engines.

Layout: rows interleaved 4-per-partition. Partition p holds padded rows
4p..4p+7 (= orig rows 4p-2..4p+5), so ALL row shifts i=0..4 are free-dim
offsets -- no cross-partition moves and no per-shift duplication.
  fin  [2, 128, 8*518] fp16: fin[img,p,row*518+col] = Fpad[img,4p+row,col+1]
  win8 [2, 128, 8*2048] fp8e4: taps (0,j*) + (1,{0,2,4}) in consumption order
  wodd [2, 128, 2*2048] fp16: taps (1,1), (1,3)
  win  [2, 3, 128, 5*4*512] fp16: tap groups 2-4
  oout [2, 128, 4*512] fp16: oout[img,p,r*512+c] = out[img,4p+r,c]
"""

import os
import sys

import numpy as np

for _p in ("/opt/trn_rl_repo",):
    if _p not in sys.path and os.path.isdir(_p):
        sys.path.insert(0, _p)

K = 5
NCORES = 8
IMGS_PER_CORE = 2
H = W = 512
RPP = 4          # output rows per partition
FROWS = RPP + K - 1  # 8 padded rows held per partition
FCOLS = 518
FH_FREE = FROWS * FCOLS  # 4144 elems per parity copy
W_FREE = K * RPP * W  # 10240
T_FREE = RPP * W  # 2048 (one tap's weights / one product / output)
KORDER = [0, 2, 4, 1, 3]  # even-j taps first within each group
# fp8 taps, in consumption order: all of group 0
FP8_TAPS = [(0, 0), (0, 2), (0, 4), (0, 1), (0, 3)]
N8 = len(FP8_TAPS)

_compiled = {}
last_results = None  # BassKernelResults of the most recent run (for test.py)


def _build_nc():
    import concourse.bacc as bacc
    import concourse.mybir as mybir
    from concourse.tile import TileContext

    f16 = mybir.dt.float16
    f32 = mybir.dt.float32
    f8 = mybir.dt.float8e4

    nc = bacc.Bacc(None, target_bir_lowering=False, debug=False)
    fin = nc.dram_tensor("fin", [IMGS_PER_CORE, 128, FH_FREE], f16,
                         kind="ExternalInput")
    win8 = nc.dram_tensor("win8", [IMGS_PER_CORE, 128, N8 * T_FREE], f8,
                          kind="ExternalInput")
    win = nc.dram_tensor("win", [IMGS_PER_CORE, K - 1, 128, W_FREE], f16,
                         kind="ExternalInput")
    iden = nc.dram_tensor("iden", [128, 128], f16, kind="ExternalInput")
    oout = nc.dram_tensor("oout", [IMGS_PER_CORE, 128, T_FREE], f16,
                          kind="ExternalOutput")

    with TileContext(nc) as tc:
        with (
            tc.tile_pool(name="idp", bufs=1) as idp,
            tc.tile_pool(name="fpool", bufs=1) as fpool,
            tc.tile_pool(name="w8tap", bufs=5) as w8tap,
            tc.tile_pool(name="w8p", bufs=1) as w8p,
            tc.tile_pool(name="wgrp", bufs=4) as wgrp,
            tc.tile_pool(name="wtap", bufs=5) as wtap,
            tc.tile_pool(name="prpool", bufs=6) as prpool,
            tc.tile_pool(name="gppool", bufs=1) as gppool,
            tc.tile_pool(name="opool", bufs=1) as opool,
            tc.psum_pool(name="ppool", bufs=1) as ppool,
        ):
            id_t = idp.tile([128, 128], f16)
            nc.scalar.dma_start(out=id_t[:], in_=iden[:])

            for img in range(IMGS_PER_CORE):
                last_img = img == IMGS_PER_CORE - 1

                f0_t = fpool.tile([128, FH_FREE], f16, tag=f"f0_{img}")
                nc.sync.dma_start(out=f0_t[:], in_=fin[img])
                # on-chip odd-parity copy: par1[c] = par0[c-1]; col 0 unread
                f1_t = fpool.tile([128, FH_FREE], f16, tag=f"f1_{img}")
                nc.scalar.copy(out=f1_t[:, 1:FH_FREE],
                               in_=f0_t[:, 0:FH_FREE - 1])
                fviews = [
                    f0_t[:].rearrange("p (row col) -> p row col", col=FCOLS),
                    f1_t[:].rearrange("p (row col) -> p row col", col=FCOLS),
                ]

                if last_img:
                    # per-bank PSUM tiles so the final casts/stores can
                    # pipeline with the last tap's matmuls
                    psb = [ppool.tile([128, W], f32, tag=f"psb{b}",
                                      name=f"psb{b}")
                           for b in range(RPP)]
                    ps_out = lambda b: psb[b][:]
                else:
                    ps = ppool.tile([128, T_FREE], f32, tag=f"ps{img}")
                    ps_out = lambda b: ps[:, b * W:(b + 1) * W]

                # ---- build the 25-tap consumption schedule ----
                # taps: list of (wview, i, j); group 0 ships as raw fp8
                # (1x DVE mode, overlapped with the stream ramp)
                taps = []
                if img == 0:
                    # per-tap fp8 chunks: compute starts ~3.5us sooner
                    for c, (i, j) in enumerate(FP8_TAPS):
                        w_t = w8tap.tile([128, T_FREE], f8, tag="w8t",
                                         name="w8t")
                        nc.sync.dma_start(
                            out=w_t[:],
                            in_=win8[img][:, c * T_FREE:(c + 1) * T_FREE])
                        taps.append((w_t[:].rearrange(
                            "p (r c) -> p r c", r=RPP), i, j))
                else:
                    w_t = w8p.tile([128, N8 * T_FREE], f8, tag="w8")
                    nc.sync.dma_start(out=w_t[:], in_=win8[img])
                    wv = w_t[:].rearrange("p (c r cc) -> p c r cc",
                                          c=N8, r=RPP, cc=W)
                    for c, (i, j) in enumerate(FP8_TAPS):
                        taps.append((wv[:, c], i, j))

                gp_prods = []
                for tg in range(1, K):
                    if last_img and tg == K - 1:
                        for k in KORDER:
                            w_t = wtap.tile([128, T_FREE], f16, tag="wt",
                                            name="wt")
                            nc.sync.dma_start(
                                out=w_t[:],
                                in_=win[img, tg - 1][:, k * T_FREE:(k + 1) * T_FREE])
                            taps.append((w_t[:].rearrange(
                                "p (r c) -> p r c", r=RPP), tg, k))
                    else:
                        w_t = wgrp.tile([128, W_FREE], f16, tag="wg")
                        nc.sync.dma_start(out=w_t[:], in_=win[img, tg - 1])
                        wv = w_t[:].rearrange("p (k r c) -> p k r c",
                                              k=K, r=RPP, c=W)
                        for k in KORDER:
                            if tg == 1 and k in (0, 2):
                                # GpSimd computes these two taps' products
                                # concurrently; their accumulation happens
                                # at the end of the image so PE never
                                # waits on the slower engine mid-stream
                                gprod = gppool.tile([128, T_FREE], f16,
                                                    tag=f"gp{img}_{k}",
                                                    name="gprod")
                                gpv = gprod[:].rearrange(
                                    "p (r c) -> p r c", r=RPP)
                                par = k & 1
                                joff = k + par
                                f_ap = fviews[par][:, tg:tg + RPP,
                                                   joff:joff + W]
                                nc.gpsimd.tensor_mul(out=gpv, in0=wv[:, k],
                                                     in1=f_ap)
                                gp_prods.append(gprod)
                            else:
                                taps.append((wv[:, k], tg, k))

                # ---- compute ----
                for n, (w_ap, i, j) in enumerate(taps):
                    par = j & 1
                    joff = j + par
                    prod = prpool.tile([128, T_FREE], f16, tag="pr")
                    pv = prod[:].rearrange("p (r c) -> p r c", r=RPP)
                    f_ap = fviews[par][:, i:i + RPP, joff:joff + W]
                    nc.vector.tensor_mul(out=pv, in0=w_ap, in1=f_ap)
                    for b in range(RPP):
                        nc.tensor.matmul(
                            ps_out(b),
                            id_t[:],
                            prod[:, b * W:(b + 1) * W],
                            start=(n == 0),
                            stop=False,
                        )
                for gi, gprod in enumerate(gp_prods):
                    for b in range(RPP):
                        nc.tensor.matmul(
                            ps_out(b),
                            id_t[:],
                            gprod[:, b * W:(b + 1) * W],
                            start=False,
                            stop=(gi == len(gp_prods) - 1),
                        )

                o_t = opool.tile([128, T_FREE], f16, tag=f"o{img}")
                if last_img:
                    # pipeline: cast banks on alternating engines as each
                    # bank's accumulation completes; store in two halves
                    # on separate DGE rings
                    for b in range(RPP):
                        ob = o_t[:, b * W:(b + 1) * W]
                        if b % 2 == 0:
                            nc.vector.tensor_copy(out=ob, in_=ps_out(b))
                        else:
                            nc.scalar.copy(out=ob, in_=ps_out(b))
                        if b == 1:
                            nc.sync.dma_start(out=oout[img][:, :2 * W],
                                              in_=o_t[:, :2 * W])
                    nc.scalar.dma_start(out=oout[img][:, 2 * W:],
                                        in_=o_t[:, 2 * W:])
                else:
                    for b in range(RPP):
                        nc.scalar.copy(out=o_t[:, b * W:(b + 1) * W],
                                       in_=ps_out(b))
                    nc.scalar.dma_start(out=oout[img], in_=o_t[:])
    nc.finalize()
    return nc


def _host_prep(frames, core):
    """Build per-core in_maps. frames [4,4,1,512,512] f32, core [4,4,25,1,512,512]."""
    import concourse.mybir as mybir

    G = NCORES * IMGS_PER_CORE  # 16
    F = np.ascontiguousarray(frames.reshape(G, H, W))
    Wc = core.reshape(G, K * K, H, W)

    # frames: Fpad[g, R, C] = F[g, R-2, C-3]; rows pad 2/2, cols 3/4
    Fp = np.pad(F, ((0, 0), (2, 2), (3, 4))).astype(np.float16)  # [G,516,519]
    # 8-row windows starting at every 4th row: sw[g, p, row, col] = Fp[g, 4p+row, col]
    sw = np.lib.stride_tricks.sliding_window_view(Fp, FROWS, axis=1)
    sw = sw[:, ::RPP].transpose(0, 1, 3, 2)  # [G, 128, 8, 519]
    fprep = np.ascontiguousarray(sw[..., 1:1 + FCOLS])  # par=0: Fpad col c+1

    # weights: wall[g, t, p, r, c] = core[g, t, 4p+r, c]
    wall = Wc.reshape(G, K * K, 128, RPP, W).transpose(0, 1, 2, 3, 4)
    wall = Wc.reshape(G, K * K, 128, RPP, W)
    f8np = mybir.dt.np(mybir.dt.float8e4)

    w8 = np.stack([wall[:, 5 * i + j] for i, j in FP8_TAPS],
                  axis=2)  # [G, 128, 5, r, c]
    w8 = w8.astype(f8np)
    w16 = (wall[:, 5:].reshape(G, K - 1, K, 128, RPP, W)
           .transpose(0, 1, 3, 2, 4, 5).astype(np.float16))

    iden = np.eye(128, dtype=np.float16)
    in_maps = []
    for c in range(NCORES):
        g0 = c * IMGS_PER_CORE
        sl = slice(g0, g0 + IMGS_PER_CORE)
        in_maps.append({
            "fin": np.ascontiguousarray(
                fprep[sl].reshape(IMGS_PER_CORE, 128, FH_FREE)),
            "win8": np.ascontiguousarray(
                w8[sl].reshape(IMGS_PER_CORE, 128, N8 * T_FREE)),
            "win": np.ascontiguousarray(
                w16[sl].reshape(IMGS_PER_CORE, K - 1, 128, W_FREE)),
            "iden": iden,
        })
    return in_maps


def kernel(frames, core, bias):
    global last_results
    from concourse.bass_utils import run_bass_kernel_spmd

    frames = np.asarray(frames, dtype=np.float32)
    core = np.asarray(core, dtype=np.float32)

    if "nc" not in _compiled:
        _compiled["nc"] = _build_nc()
    nc = _compiled["nc"]

    in_maps = _host_prep(frames, core)
    trace = os.environ.get("KC_TRACE") == "1"
    tmpdir = os.environ.get("KC_TRACE_DIR") or None
    if tmpdir:
        os.makedirs(tmpdir, exist_ok=True)
    res = run_bass_kernel_spmd(nc, in_maps, list(range(NCORES)), trace=trace,
                               tmpdir=tmpdir)
    last_results = res

    G = NCORES * IMGS_PER_CORE
    out = np.empty((G, H, W), np.float32)
    for c in range(NCORES):
        o = res.results[c]["oout"]  # [2, 128, 2048] f16; rows are 4p+r in order
        for img in range(IMGS_PER_CORE):
            out[c * IMGS_PER_CORE + img] = o[img].reshape(H, W).astype(np.float32)
    return out.reshape(4, 4, H, W)


# revision 16
# speedup vs baseline: 1.0505x; 1.0505x over previous
"""Per-pixel adaptive 5x5 conv (KPN) for Trainium2, 8-core data parallel.

out[g,h,w] = sum_{i,j} core[g,5i+j,h,w] * frames_pad[g,h+i-2,w+j-2]
with g = flattened (B,N) = 16 image planes; 2 planes per NeuronCore.

Engine split (v8):
  DVE    : the 25 per-tap multiplies per image, ALL in fp16 2x_1P mode,
           plus half of the last image's PSUM casts.
  TensorE: all tap accumulation, via identity-matmul into PSUM (fp32
           accumulate; 4 matmuls of FD=512 per tap = 1 PSUM bank each).
  ScalarE: builds the odd-parity frame copy on-chip (saves 2.1 MB of HBM
           per core), img0's PSUM casts, half the output stores.
  GpSimd : idle (shares SBUF ports with DVE; using it slows DVE).
           5 of 25 tap planes per image ship as raw fp8-e4m3 (halves
           their HBM bytes; those DVE muls run 1x). rel err ~1.2e-2.

The end-to-end time is paced by DVE (25 x 1.22us muls per image) vs the
HBM weight stream through 16 SDMA engines, one of which runs ~17% slow
and gates every completion semaphore; fp8 shipping + the on-chip parity
copy + fp16 outputs cut the stream to ~24 MB/core to rebalance. First
fp8 group streams per-tap (compute starts ~12.5us), last fp16 group
streams per-tap (post-stream tail is one tap), final casts/stores
pipeline with the last taps' matmuls on alternating engines.

Layout: rows interleaved 4-per-partition. Partition p holds padded rows
4p..4p+7 (= orig rows 4p-2..4p+5), so ALL row shifts i=0..4 are free-dim
offsets -- no cross-partition moves and no per-shift duplication.
  fin  [2, 128, 8*518] fp16: fin[img,p,row*518+col] = Fpad[img,4p+row,col+1]
  win8 [2, 128, 8*2048] fp8e4: taps (0,j*) + (1,{0,2,4}) in consumption order
  wodd [2, 128, 2*2048] fp16: taps (1,1), (1,3)
  win  [2, 3, 128, 5*4*512] fp16: tap groups 2-4
  oout [2, 128, 4*512] fp16: oout[img,p,r*512+c] = out[img,4p+r,c]
"""

import os
import sys

import numpy as np

for _p in ("/opt/trn_rl_repo",):
    if _p not in sys.path and os.path.isdir(_p):
        sys.path.insert(0, _p)

K = 5
NCORES = 8
IMGS_PER_CORE = 2
H = W = 512
RPP = 4          # output rows per partition
FROWS = RPP + K - 1  # 8 padded rows held per partition
FCOLS = 518
FH_FREE = FROWS * FCOLS  # 4144 elems per parity copy
W_FREE = K * RPP * W  # 10240
T_FREE = RPP * W  # 2048 (one tap's weights / one product / output)
KORDER = [0, 2, 4, 1, 3]  # even-j taps first within each group
# fp8 taps, in consumption order: all of group 0
FP8_TAPS = [(0, 0), (0, 2), (0, 4), (0, 1), (0, 3)]
N8 = len(FP8_TAPS)

_compiled = {}
last_results = None  # BassKernelResults of the most recent run (for test.py)


def _build_nc():
    import concourse.bacc as bacc
    import concourse.mybir as mybir
    from concourse.tile import TileContext

    f16 = mybir.dt.float16
    f32 = mybir.dt.float32
    f8 = mybir.dt.float8e4

    nc = bacc.Bacc(None, target_bir_lowering=False, debug=False)
    fin = nc.dram_tensor("fin", [IMGS_PER_CORE, 128, FH_FREE], f16,
                         kind="ExternalInput")
    win8 = nc.dram_tensor("win8", [IMGS_PER_CORE, 128, N8 * T_FREE], f8,
                          kind="ExternalInput")
    win = nc.dram_tensor("win", [IMGS_PER_CORE, K - 1, 128, W_FREE], f16,
                         kind="ExternalInput")
    iden = nc.dram_tensor("iden", [128, 128], f16, kind="ExternalInput")
    oout = nc.dram_tensor("oout", [IMGS_PER_CORE, 128, T_FREE], f16,
                          kind="ExternalOutput")

    with TileContext(nc) as tc:
        with (
            tc.tile_pool(name="idp", bufs=1) as idp,
            tc.tile_pool(name="fpool", bufs=1) as fpool,
            tc.tile_pool(name="w8tap", bufs=5) as w8tap,
            tc.tile_pool(name="w8p", bufs=1) as w8p,
            tc.tile_pool(name="wgrp", bufs=4) as wgrp,
            tc.tile_pool(name="wtap", bufs=5) as wtap,
            tc.tile_pool(name="prpool", bufs=6) as prpool,
            tc.tile_pool(name="opool", bufs=1) as opool,
            tc.psum_pool(name="ppool", bufs=1) as ppool,
        ):
            id_t = idp.tile([128, 128], f16)
            nc.scalar.dma_start(out=id_t[:], in_=iden[:])

            for img in range(IMGS_PER_CORE):
                last_img = img == IMGS_PER_CORE - 1

                f0_t = fpool.tile([128, FH_FREE], f16, tag=f"f0_{img}")
                nc.sync.dma_start(out=f0_t[:], in_=fin[img])
                # on-chip odd-parity copy: par1[c] = par0[c-1]; col 0 unread
                f1_t = fpool.tile([128, FH_FREE], f16, tag=f"f1_{img}")
                nc.scalar.copy(out=f1_t[:, 1:FH_FREE],
                               in_=f0_t[:, 0:FH_FREE - 1])
                fviews = [
                    f0_t[:].rearrange("p (row col) -> p row col", col=FCOLS),
                    f1_t[:].rearrange("p (row col) -> p row col", col=FCOLS),
                ]

                if last_img:
                    # per-bank PSUM tiles so the final casts/stores can
                    # pipeline with the last tap's matmuls
                    psb = [ppool.tile([128, W], f32, tag=f"psb{b}",
                                      name=f"psb{b}")
                           for b in range(RPP)]
                    ps_out = lambda b: psb[b][:]
                else:
                    ps = ppool.tile([128, T_FREE], f32, tag=f"ps{img}")
                    ps_out = lambda b: ps[:, b * W:(b + 1) * W]

                # ---- build the 25-tap consumption schedule ----
                # taps: list of (wview, i, j); group 0 ships as raw fp8
                # (1x DVE mode, overlapped with the stream ramp)
                taps = []
                if img == 0:
                    # per-tap fp8 chunks: compute starts ~3.5us sooner
                    for c, (i, j) in enumerate(FP8_TAPS):
                        w_t = w8tap.tile([128, T_FREE], f8, tag="w8t",
                                         name="w8t")
                        nc.sync.dma_start(
                            out=w_t[:],
                            in_=win8[img][:, c * T_FREE:(c + 1) * T_FREE])
                        taps.append((w_t[:].rearrange(
                            "p (r c) -> p r c", r=RPP), i, j))
                else:
                    w_t = w8p.tile([128, N8 * T_FREE], f8, tag="w8")
                    nc.sync.dma_start(out=w_t[:], in_=win8[img])
                    wv = w_t[:].rearrange("p (c r cc) -> p c r cc",
                                          c=N8, r=RPP, cc=W)
                    for c, (i, j) in enumerate(FP8_TAPS):
                        taps.append((wv[:, c], i, j))

                for tg in range(1, K):
                    if last_img and tg == K - 1:
                        for k in KORDER:
                            w_t = wtap.tile([128, T_FREE], f16, tag="wt",
                                            name="wt")
                            nc.sync.dma_start(
                                out=w_t[:],
                                in_=win[img, tg - 1][:, k * T_FREE:(k + 1) * T_FREE])
                            taps.append((w_t[:].rearrange(
                                "p (r c) -> p r c", r=RPP), tg, k))
                    else:
                        w_t = wgrp.tile([128, W_FREE], f16, tag="wg")
                        nc.sync.dma_start(out=w_t[:], in_=win[img, tg - 1])
                        wv = w_t[:].rearrange("p (k r c) -> p k r c",
                                              k=K, r=RPP, c=W)
                        for k in KORDER:
                            taps.append((wv[:, k], tg, k))

                # ---- compute ----
                for n, (w_ap, i, j) in enumerate(taps):
                    par = j & 1
                    joff = j + par
                    prod = prpool.tile([128, T_FREE], f16, tag="pr")
                    pv = prod[:].rearrange("p (r c) -> p r c", r=RPP)
                    f_ap = fviews[par][:, i:i + RPP, joff:joff + W]
                    nc.vector.tensor_mul(out=pv, in0=w_ap, in1=f_ap)
                    for b in range(RPP):
                        nc.tensor.matmul(
                            ps_out(b),
                            id_t[:],
                            prod[:, b * W:(b + 1) * W],
                            start=(n == 0),
                            stop=(n == K * K - 1),
                        )

                o_t = opool.tile([128, T_FREE], f16, tag=f"o{img}")
                if last_img:
                    # pipeline: cast banks on alternating engines as each
                    # bank's accumulation completes; store in two halves
                    # on separate DGE rings
                    for b in range(RPP):
                        ob = o_t[:, b * W:(b + 1) * W]
                        if b % 2 == 0:
                            nc.vector.tensor_copy(out=ob, in_=ps_out(b))
                        else:
                            nc.scalar.copy(out=ob, in_=ps_out(b))
                        if b == 1:
                            nc.sync.dma_start(out=oout[img][:, :2 * W],
                                              in_=o_t[:, :2 * W])
                    nc.scalar.dma_start(out=oout[img][:, 2 * W:],
                                        in_=o_t[:, 2 * W:])
                else:
                    for b in range(RPP):
                        nc.scalar.copy(out=o_t[:, b * W:(b + 1) * W],
                                       in_=ps_out(b))
                    nc.scalar.dma_start(out=oout[img], in_=o_t[:])
    nc.finalize()
    return nc


def _host_prep(frames, core):
    """Build per-core in_maps. frames [4,4,1,512,512] f32, core [4,4,25,1,512,512]."""
    import concourse.mybir as mybir

    G = NCORES * IMGS_PER_CORE  # 16
    F = np.ascontiguousarray(frames.reshape(G, H, W))
    Wc = core.reshape(G, K * K, H, W)

    # frames: Fpad[g, R, C] = F[g, R-2, C-3]; rows pad 2/2, cols 3/4
    Fp = np.pad(F, ((0, 0), (2, 2), (3, 4))).astype(np.float16)  # [G,516,519]
    # 8-row windows starting at every 4th row: sw[g, p, row, col] = Fp[g, 4p+row, col]
    sw = np.lib.stride_tricks.sliding_window_view(Fp, FROWS, axis=1)
    sw = sw[:, ::RPP].transpose(0, 1, 3, 2)  # [G, 128, 8, 519]
    fprep = np.ascontiguousarray(sw[..., 1:1 + FCOLS])  # par=0: Fpad col c+1

    # weights: wall[g, t, p, r, c] = core[g, t, 4p+r, c]
    wall = Wc.reshape(G, K * K, 128, RPP, W).transpose(0, 1, 2, 3, 4)
    wall = Wc.reshape(G, K * K, 128, RPP, W)
    f8np = mybir.dt.np(mybir.dt.float8e4)

    w8 = np.stack([wall[:, 5 * i + j] for i, j in FP8_TAPS],
                  axis=2)  # [G, 128, 5, r, c]
    w8 = w8.astype(f8np)
    w16 = (wall[:, 5:].reshape(G, K - 1, K, 128, RPP, W)
           .transpose(0, 1, 3, 2, 4, 5).astype(np.float16))

    iden = np.eye(128, dtype=np.float16)
    in_maps = []
    for c in range(NCORES):
        g0 = c * IMGS_PER_CORE
        sl = slice(g0, g0 + IMGS_PER_CORE)
        in_maps.append({
            "fin": np.ascontiguousarray(
                fprep[sl].reshape(IMGS_PER_CORE, 128, FH_FREE)),
            "win8": np.ascontiguousarray(
                w8[sl].reshape(IMGS_PER_CORE, 128, N8 * T_FREE)),
            "win": np.ascontiguousarray(
                w16[sl].reshape(IMGS_PER_CORE, K - 1, 128, W_FREE)),
            "iden": iden,
        })
    return in_maps


def kernel(frames, core, bias):
    global last_results
    from concourse.bass_utils import run_bass_kernel_spmd

    frames = np.asarray(frames, dtype=np.float32)
    core = np.asarray(core, dtype=np.float32)

    if "nc" not in _compiled:
        _compiled["nc"] = _build_nc()
    nc = _compiled["nc"]

    in_maps = _host_prep(frames, core)
    trace = os.environ.get("KC_TRACE") == "1"
    tmpdir = os.environ.get("KC_TRACE_DIR") or None
    if tmpdir:
        os.makedirs(tmpdir, exist_ok=True)
    res = run_bass_kernel_spmd(nc, in_maps, list(range(NCORES)), trace=trace,
                               tmpdir=tmpdir)
    last_results = res

    G = NCORES * IMGS_PER_CORE
    out = np.empty((G, H, W), np.float32)
    for c in range(NCORES):
        o = res.results[c]["oout"]  # [2, 128, 2048] f16; rows are 4p+r in order
        for img in range(IMGS_PER_CORE):
            out[c * IMGS_PER_CORE + img] = o[img].reshape(H, W).astype(np.float32)
    return out.reshape(4, 4, H, W)


# revision 18
# speedup vs baseline: 1.0600x; 1.0091x over previous
"""Per-pixel adaptive 5x5 conv (KPN) for Trainium2, 8-core data parallel.

out[g,h,w] = sum_{i,j} core[g,5i+j,h,w] * frames_pad[g,h+i-2,w+j-2]
with g = flattened (B,N) = 16 image planes; 2 planes per NeuronCore.

Engine split (v8):
  DVE    : the 25 per-tap multiplies per image, ALL in fp16 2x_1P mode,
           plus half of the last image's PSUM casts.
  TensorE: all tap accumulation, via identity-matmul into PSUM (fp32
           accumulate; 4 matmuls of FD=512 per tap = 1 PSUM bank each).
  ScalarE: builds the odd-parity frame copy on-chip (saves 2.1 MB of HBM
           per core), img0's PSUM casts, half the output stores.
  GpSimd : idle (shares SBUF ports with DVE; using it slows DVE).
           5 of 25 tap planes per image ship as raw fp8-e4m3 (halves
           their HBM bytes; those DVE muls run 1x). rel err ~1.2e-2.

The end-to-end time is paced by DVE (25 x 1.22us muls per image) vs the
HBM weight stream through 16 SDMA engines, one of which runs ~17% slow
and gates every completion semaphore; fp8 shipping + the on-chip parity
copy + fp16 outputs cut the stream to ~24 MB/core to rebalance. First
fp8 group streams per-tap (compute starts ~12.5us), last fp16 group
streams per-tap (post-stream tail is one tap), final casts/stores
pipeline with the last taps' matmuls on alternating engines.

Layout: rows interleaved 4-per-partition. Partition p holds padded rows
4p..4p+7 (= orig rows 4p-2..4p+5), so ALL row shifts i=0..4 are free-dim
offsets -- no cross-partition moves and no per-shift duplication.
  fin  [2, 128, 8*518] fp16: fin[img,p,row*518+col] = Fpad[img,4p+row,col+1]
  win8 [2, 128, 8*2048] fp8e4: taps (0,j*) + (1,{0,2,4}) in consumption order
  wodd [2, 128, 2*2048] fp16: taps (1,1), (1,3)
  win  [2, 3, 128, 5*4*512] fp16: tap groups 2-4
  oout [2, 128, 4*512] fp16: oout[img,p,r*512+c] = out[img,4p+r,c]
"""

import os
import sys

import numpy as np

for _p in ("/opt/trn_rl_repo",):
    if _p not in sys.path and os.path.isdir(_p):
        sys.path.insert(0, _p)

K = 5
NCORES = 8
IMGS_PER_CORE = 2
H = W = 512
RPP = 4          # output rows per partition
FROWS = RPP + K - 1  # 8 padded rows held per partition
FCOLS = 518
FH_FREE = FROWS * FCOLS  # 4144 elems per parity copy
W_FREE = K * RPP * W  # 10240
T_FREE = RPP * W  # 2048 (one tap's weights / one product / output)
KORDER = [0, 2, 4, 1, 3]  # even-j taps first within each group
# fp8 taps, in consumption order: all of group 0, plus (1,0)
FP8_TAPS = [(0, 0), (0, 2), (0, 4), (0, 1), (0, 3), (1, 0)]
N8 = len(FP8_TAPS)
KORDER1 = [2, 4, 1, 3]  # group-1 fp16 taps (j=0 ships as fp8)

_compiled = {}
last_results = None  # BassKernelResults of the most recent run (for test.py)


def _build_nc():
    import concourse.bacc as bacc
    import concourse.mybir as mybir
    from concourse.tile import TileContext

    f16 = mybir.dt.float16
    f32 = mybir.dt.float32
    f8 = mybir.dt.float8e4

    nc = bacc.Bacc(None, target_bir_lowering=False, debug=False)
    fin = nc.dram_tensor("fin", [IMGS_PER_CORE, 128, FH_FREE], f16,
                         kind="ExternalInput")
    win8 = nc.dram_tensor("win8", [IMGS_PER_CORE, 128, N8 * T_FREE], f8,
                          kind="ExternalInput")
    wt1 = nc.dram_tensor("wt1", [IMGS_PER_CORE, 128, 4 * T_FREE], f16,
                         kind="ExternalInput")
    win = nc.dram_tensor("win", [IMGS_PER_CORE, K - 2, 128, W_FREE], f16,
                         kind="ExternalInput")
    iden = nc.dram_tensor("iden", [128, 128], f16, kind="ExternalInput")
    oout = nc.dram_tensor("oout", [IMGS_PER_CORE, 128, T_FREE], f16,
                          kind="ExternalOutput")

    with TileContext(nc) as tc:
        with (
            tc.tile_pool(name="idp", bufs=1) as idp,
            tc.tile_pool(name="fpool", bufs=1) as fpool,
            tc.tile_pool(name="w8tap", bufs=5) as w8tap,
            tc.tile_pool(name="w8p", bufs=1) as w8p,
            tc.tile_pool(name="wgrp", bufs=3) as wgrp,
            tc.tile_pool(name="wtap", bufs=5) as wtap,
            tc.tile_pool(name="wt1p", bufs=2) as wt1p,
            tc.tile_pool(name="prpool", bufs=6) as prpool,
            tc.tile_pool(name="opool", bufs=1) as opool,
            tc.psum_pool(name="ppool", bufs=1) as ppool,
        ):
            id_t = idp.tile([128, 128], f16)
            nc.scalar.dma_start(out=id_t[:], in_=iden[:])

            for img in range(IMGS_PER_CORE):
                last_img = img == IMGS_PER_CORE - 1

                f0_t = fpool.tile([128, FH_FREE], f16, tag=f"f0_{img}")
                nc.sync.dma_start(out=f0_t[:], in_=fin[img])
                # on-chip odd-parity copy: par1[c] = par0[c-1]; col 0 unread
                f1_t = fpool.tile([128, FH_FREE], f16, tag=f"f1_{img}")
                nc.scalar.copy(out=f1_t[:, 1:FH_FREE],
                               in_=f0_t[:, 0:FH_FREE - 1])
                fviews = [
                    f0_t[:].rearrange("p (row col) -> p row col", col=FCOLS),
                    f1_t[:].rearrange("p (row col) -> p row col", col=FCOLS),
                ]

                if last_img:
                    # per-bank PSUM tiles so the final casts/stores can
                    # pipeline with the last tap's matmuls
                    psb = [ppool.tile([128, W], f32, tag=f"psb{b}",
                                      name=f"psb{b}")
                           for b in range(RPP)]
                    ps_out = lambda b: psb[b][:]
                else:
                    ps = ppool.tile([128, T_FREE], f32, tag=f"ps{img}")
                    ps_out = lambda b: ps[:, b * W:(b + 1) * W]

                # ---- build the 25-tap consumption schedule ----
                # taps: list of (wview, i, j); group 0 ships as raw fp8
                # (1x DVE mode, overlapped with the stream ramp)
                taps = []
                if img == 0:
                    # per-tap fp8 chunks: compute starts ~3.5us sooner
                    for c, (i, j) in enumerate(FP8_TAPS):
                        w_t = w8tap.tile([128, T_FREE], f8, tag="w8t",
                                         name="w8t")
                        nc.sync.dma_start(
                            out=w_t[:],
                            in_=win8[img][:, c * T_FREE:(c + 1) * T_FREE])
                        taps.append((w_t[:].rearrange(
                            "p (r c) -> p r c", r=RPP), i, j))
                else:
                    w_t = w8p.tile([128, N8 * T_FREE], f8, tag="w8")
                    nc.sync.dma_start(out=w_t[:], in_=win8[img])
                    wv = w_t[:].rearrange("p (c r cc) -> p c r cc",
                                          c=N8, r=RPP, cc=W)
                    for c, (i, j) in enumerate(FP8_TAPS):
                        taps.append((wv[:, c], i, j))

                # group 1: j=0 already in win8; j 2,4,1,3 from wt1
                w1_t = wt1p.tile([128, 4 * T_FREE], f16, tag="w1")
                nc.sync.dma_start(out=w1_t[:], in_=wt1[img])
                w1v = w1_t[:].rearrange("p (k r c) -> p k r c",
                                        k=4, r=RPP, c=W)
                for ki, k in enumerate(KORDER1):
                    taps.append((w1v[:, ki], 1, k))

                for tg in range(2, K):
                    if last_img and tg == K - 1:
                        for k in KORDER:
                            w_t = wtap.tile([128, T_FREE], f16, tag="wt",
                                            name="wt")
                            nc.sync.dma_start(
                                out=w_t[:],
                                in_=win[img, tg - 2][:, k * T_FREE:(k + 1) * T_FREE])
                            taps.append((w_t[:].rearrange(
                                "p (r c) -> p r c", r=RPP), tg, k))
                    else:
                        w_t = wgrp.tile([128, W_FREE], f16, tag="wg")
                        nc.sync.dma_start(out=w_t[:], in_=win[img, tg - 2])
                        wv = w_t[:].rearrange("p (k r c) -> p k r c",
                                              k=K, r=RPP, c=W)
                        for k in KORDER:
                            taps.append((wv[:, k], tg, k))

                # ---- compute ----
                for n, (w_ap, i, j) in enumerate(taps):
                    par = j & 1
                    joff = j + par
                    prod = prpool.tile([128, T_FREE], f16, tag="pr")
                    pv = prod[:].rearrange("p (r c) -> p r c", r=RPP)
                    f_ap = fviews[par][:, i:i + RPP, joff:joff + W]
                    nc.vector.tensor_mul(out=pv, in0=w_ap, in1=f_ap)
                    for b in range(RPP):
                        nc.tensor.matmul(
                            ps_out(b),
                            id_t[:],
                            prod[:, b * W:(b + 1) * W],
                            start=(n == 0),
                            stop=(n == K * K - 1),
                        )

                o_t = opool.tile([128, T_FREE], f16, tag=f"o{img}")
                if last_img:
                    # pipeline: cast banks on alternating engines as each
                    # bank's accumulation completes; store in two halves
                    # on separate DGE rings
                    for b in range(RPP):
                        ob = o_t[:, b * W:(b + 1) * W]
                        if b % 2 == 0:
                            nc.vector.tensor_copy(out=ob, in_=ps_out(b))
                        else:
                            nc.scalar.copy(out=ob, in_=ps_out(b))
                        if b == 1:
                            nc.sync.dma_start(out=oout[img][:, :2 * W],
                                              in_=o_t[:, :2 * W])
                    nc.scalar.dma_start(out=oout[img][:, 2 * W:],
                                        in_=o_t[:, 2 * W:])
                else:
                    for b in range(RPP):
                        nc.scalar.copy(out=o_t[:, b * W:(b + 1) * W],
                                       in_=ps_out(b))
                    nc.scalar.dma_start(out=oout[img], in_=o_t[:])
    nc.finalize()
    return nc


def _host_prep(frames, core):
    """Build per-core in_maps. frames [4,4,1,512,512] f32, core [4,4,25,1,512,512]."""
    import concourse.mybir as mybir

    G = NCORES * IMGS_PER_CORE  # 16
    F = np.ascontiguousarray(frames.reshape(G, H, W))
    Wc = core.reshape(G, K * K, H, W)

    # frames: Fpad[g, R, C] = F[g, R-2, C-3]; rows pad 2/2, cols 3/4
    Fp = np.pad(F, ((0, 0), (2, 2), (3, 4))).astype(np.float16)  # [G,516,519]
    # 8-row windows starting at every 4th row: sw[g, p, row, col] = Fp[g, 4p+row, col]
    sw = np.lib.stride_tricks.sliding_window_view(Fp, FROWS, axis=1)
    sw = sw[:, ::RPP].transpose(0, 1, 3, 2)  # [G, 128, 8, 519]
    fprep = np.ascontiguousarray(sw[..., 1:1 + FCOLS])  # par=0: Fpad col c+1

    # weights: wall[g, t, p, r, c] = core[g, t, 4p+r, c]
    wall = Wc.reshape(G, K * K, 128, RPP, W).transpose(0, 1, 2, 3, 4)
    wall = Wc.reshape(G, K * K, 128, RPP, W)
    f8np = mybir.dt.np(mybir.dt.float8e4)

    w8 = np.stack([wall[:, 5 * i + j] for i, j in FP8_TAPS],
                  axis=2)  # [G, 128, 6, r, c]
    w8 = w8.astype(f8np)
    wt1h = np.stack([wall[:, 5 + j] for j in [2, 4, 1, 3]],
                    axis=2).astype(np.float16)  # [G, 128, 4, r, c]
    w16 = (wall[:, 10:].reshape(G, K - 2, K, 128, RPP, W)
           .transpose(0, 1, 3, 2, 4, 5).astype(np.float16))

    iden = np.eye(128, dtype=np.float16)
    in_maps = []
    for c in range(NCORES):
        g0 = c * IMGS_PER_CORE
        sl = slice(g0, g0 + IMGS_PER_CORE)
        in_maps.append({
            "fin": np.ascontiguousarray(
                fprep[sl].reshape(IMGS_PER_CORE, 128, FH_FREE)),
            "win8": np.ascontiguousarray(
                w8[sl].reshape(IMGS_PER_CORE, 128, N8 * T_FREE)),
            "wt1": np.ascontiguousarray(
                wt1h[sl].reshape(IMGS_PER_CORE, 128, 4 * T_FREE)),
            "win": np.ascontiguousarray(
                w16[sl].reshape(IMGS_PER_CORE, K - 2, 128, W_FREE)),
            "iden": iden,
        })
    return in_maps


def kernel(frames, core, bias):
    global last_results
    from concourse.bass_utils import run_bass_kernel_spmd

    frames = np.asarray(frames, dtype=np.float32)
    core = np.asarray(core, dtype=np.float32)

    if "nc" not in _compiled:
        _compiled["nc"] = _build_nc()
    nc = _compiled["nc"]

    in_maps = _host_prep(frames, core)
    trace = os.environ.get("KC_TRACE") == "1"
    tmpdir = os.environ.get("KC_TRACE_DIR") or None
    if tmpdir:
        os.makedirs(tmpdir, exist_ok=True)
    res = run_bass_kernel_spmd(nc, in_maps, list(range(NCORES)), trace=trace,
                               tmpdir=tmpdir)
    last_results = res

    G = NCORES * IMGS_PER_CORE
    out = np.empty((G, H, W), np.float32)
    for c in range(NCORES):
        o = res.results[c]["oout"]  # [2, 128, 2048] f16; rows are 4p+r in order
        for img in range(IMGS_PER_CORE):
            out[c * IMGS_PER_CORE + img] = o[img].reshape(H, W).astype(np.float32)
    return out.reshape(4, 4, H, W)
